# revision 33
# baseline (speedup 1.0000x reference)
"""BiMambaBlock Trainium2 kernel.

Strategy: data-parallel over batch (8 batches -> 8 NeuronCores). Each core
computes the full bidirectional Mamba block for its batch in a single Bass
program:

  - layout for the middle section: [d_inner on partitions, time on free]
  - projections (in_proj / x_proj / dt_proj / out_proj) as PE GEMMs (bf16)
  - causal depthwise conv: shifted tensor_scalar taps on DVE + adds on GPSIMD
  - selective scan via DVE tensor_tensor_scan (state = dA*state + dBx), one
    lane per (d, n) pair; backward direction scans reversed-time APs
  - dBx/hC elementwise multiplies split ~80/20 between GPSIMD and DVE
  - n-fold (sum_n C_n * h_n) via identity-matmul PSUM accumulation on PE
  - final combine + layernorm in [time on partitions, d_model on free]

Engine budget per core (cost model): DVE ~340us, Pool ~360us, ACT ~340us,
PE ~290us, SP ~290us. Phases ordered A(fwd), A(bwd), BCD(fwd), BCD(bwd) so
the ACT table set switches only once (sigmoid set -> exp/ln set).
"""

import sys

sys.path.insert(0, "/opt/trn_rl_repo")

import numpy as np

import concourse.bass as bass
import concourse.mybir as mybir
import concourse.tile as tile
from concourse import bacc
from concourse.bass_utils import run_bass_kernel_spmd

import ml_dtypes

F32 = mybir.dt.float32
BF16 = mybir.dt.bfloat16
AF = mybir.ActivationFunctionType
OP = mybir.AluOpType

B, L, D, DI, NST, RNK, KCONV = 8, 1024, 512, 1024, 16, 32, 4
LN_EPS = 1e-5
NB = DI // 128  # 8 d-blocks
TT = L // 128  # 8 time tiles
TCH = L // 512  # 2 matmul free chunks
PAD = KCONV - 1

POOL_SCAN = False  # TensorTensorScanArith is not a legal Pool opcode (walrus ISA check)
# DVE and GPSIMD share SBUF read/write ports: overlapping them throttles BOTH
# ~2x (measured). DVE in 2x bf16 mode moves the same elements in ~1/3 the port
# time of GPSIMD (0.42 impl efficiency), so phase C routes everything to DVE
# and leaves GPSIMD idle.
POOL_DBX = False

nbf = ml_dtypes.bfloat16


class P:
    """Pool/handle bag shared by the phase builders."""


def _load_dir_consts(nc, p, cst, pre):
    s_pool = p.s_pool
    h = {}
    h["conv_w"] = [s_pool.tile([128, KCONV], F32, tag=f"conv_w{m}", name=f"conv_w{m}") for m in range(NB)]
    h["conv_b"] = [s_pool.tile([128, 1], F32, tag=f"conv_b{m}", name=f"conv_b{m}") for m in range(NB)]
    h["dt_b"] = [s_pool.tile([128, 1], F32, tag=f"dt_b{m}", name=f"dt_b{m}") for m in range(NB)]
    h["a_sb"] = [s_pool.tile([128, NST], F32, tag=f"a_sb{m}", name=f"a_sb{m}") for m in range(NB)]
    h["dv"] = [s_pool.tile([128, 1], F32, tag=f"dv{m}", name=f"dv{m}") for m in range(NB)]
    for m in range(NB):
        sl = slice(128 * m, 128 * (m + 1))
        nc.sync.dma_start(h["conv_w"][m][:], cst[pre + "conv_w"][sl, :])
        nc.sync.dma_start(h["conv_b"][m][:], cst[pre + "conv_b"][sl, :])
        nc.sync.dma_start(h["dt_b"][m][:], cst[pre + "dt_b"][sl, :])
        nc.sync.dma_start(h["a_sb"][m][:], cst[pre + "A"][sl, :])
        nc.sync.dma_start(h["dv"][m][:], cst[pre + "Dv"][sl, :])
    return h


def _sil(p, inst):
    """Register a silu-table ACT op; order it after any prior exp/ln epoch."""
    p.sig_insts.append(inst)
    for ei in p.exp_insts:
        bass._add_dep_helper(inst.ins, ei.ins, sync=False, reason="act-table-epoch")


def _phase_a(nc, p, cst, pre, rev):
    """in_proj GEMM; z -> silu(z); xi -> causal conv -> silu -> xc."""
    w_in = [p.w_pool.tile([128, 2 * DI], BF16, tag=f"w_in{k}", name=f"w_in{k}") for k in range(4)]
    for k in range(4):
        nc.sync.dma_start(w_in[k][:], cst[pre + "w_inT"][128 * k:128 * (k + 1), :])

    xc = [p.big_pool.tile([128, L], BF16, tag=f"{pre}xc{m}", name=f"{pre}xc{m}") for m in range(NB)]
    siluz = [p.big_pool.tile([128, L], BF16, tag=f"{pre}sz{m}", name=f"{pre}sz{m}") for m in range(NB)]

    # xi tiles first so DVE conv work starts as early as possible (silu-z is
    # pure ACT now, so z tiles can come second; all Silu still precede Exp)
    for m in range(2 * NB):
        mm = m
        xi_pad = None
        if mm < NB:
            xi_pad = p.work_pool.tile([128, L + PAD], BF16, tag="xi_pad", name="xi_pad", bufs=2)
            if rev:
                nc.vector.memset(xi_pad[:, L:L + PAD], 0.0)
            else:
                nc.vector.memset(xi_pad[:, 0:PAD], 0.0)
        for tch in range(TCH):
            ps = p.ps_pool.tile([128, 512], F32, tag="mm", name="mm")
            for k in range(4):
                nc.tensor.matmul(
                    ps[:],
                    w_in[k][:, 128 * mm:128 * (mm + 1)],
                    p.xT[k][:, 512 * tch:512 * (tch + 1)],
                    start=(k == 0),
                    stop=(k == 3),
                )
            if mm < NB:
                off = (0 if rev else PAD) + 512 * tch
                nc.scalar.activation(xi_pad[:, off:off + 512], ps[:], AF.Copy)
            else:
                # silu(z) in one ACT op straight from PSUM (silu table set)
                _sil(p, nc.scalar.activation(
                    siluz[mm - NB][:, 512 * tch:512 * (tch + 1)], ps[:], AF.Silu))
        if mm < NB:
            # conv: fwd out[t] = sum_j w_j*xi[t-3+j]; bwd out[t] = sum_j w_j*xi[t+3-j]
            acc = p.work_pool.tile([128, L], BF16, tag="cacc", name="cacc", bufs=1)
            cw = _phase_a.consts[pre]["conv_w"][mm]
            cb = _phase_a.consts[pre]["conv_b"][mm]
            offs = [3 - j for j in range(KCONV)] if rev else list(range(KCONV))
            taps = []
            for j in range(KCONV):
                o = offs[j]
                tp = p.work_pool.tile(
                    [128, L], BF16, tag=["da", "dbx", "spu", "hc"][j],
                    name=f"tap{j}", bufs=1 if j == 2 else 2)
                nc.vector.tensor_scalar(tp[:], xi_pad[:, o:o + L], cw[:, j:j + 1], None, OP.mult)
                taps.append(tp)
            nc.vector.tensor_tensor(taps[0][:], taps[0][:], taps[1][:], OP.add)
            nc.vector.tensor_tensor(taps[2][:], taps[2][:], taps[3][:], OP.add)
            nc.vector.tensor_tensor(acc[:], taps[0][:], taps[2][:], OP.add)
            # xc = silu(acc + conv_b) in one ACT op
            _sil(p, nc.scalar.activation(xc[mm][:], acc[:], AF.Silu, bias=cb[:, 0:1]))
    return {"xc": xc, "siluz": siluz}


_phase_a.consts = {}


def _exp(p, inst):
    """Register an exp/ln-table ACT op; order it after all prior silu ops."""
    p.exp_insts.append(inst)
    for si in p.sig_insts:
        bass._add_dep_helper(inst.ins, si.ins, sync=False, reason="act-table-epoch")
    return inst


def _phase_b(nc, p, cst, pre, ten):
    """x_proj -> (dt | B | C) -> bc_dram; dt_proj -> softplus -> delta."""
    xc = ten["xc"]
    con = _phase_a.consts[pre]
    w_x = [p.w_pool.tile([128, 64], BF16, tag=f"w_x{k}", name=f"w_x{k}") for k in range(NB)]
    for k in range(NB):
        nc.sync.dma_start(w_x[k][:], cst[pre + "w_xT"][128 * k:128 * (k + 1), :])
    w_dt = p.w_pool.tile([RNK, DI], BF16, tag="w_dt", name="w_dt")
    nc.sync.dma_start(w_dt[:], cst[pre + "w_dtT"][:])

    dbl = p.big_pool.tile([64, L], BF16, tag="dbl", name="dbl")
    for tch in range(TCH):
        ps = p.ps_pool.tile([64, 512], F32, tag="mm", name="mm")
        for k in range(NB):
            nc.tensor.matmul(
                ps[:], w_x[k][:], xc[k][:, 512 * tch:512 * (tch + 1)],
                start=(k == 0), stop=(k == NB - 1),
            )
        nc.scalar.activation(dbl[:, 512 * tch:512 * (tch + 1)], ps[:], AF.Copy)
    bc_dram = p.dram_pool.tile([2 * NST, L], BF16, tag="bc_dram", name="bc_dram", bufs=2)
    nc.sync.dma_start(bc_dram[:], dbl[RNK:RNK + 2 * NST, :])

    delta = [p.big_pool.tile([128, L], BF16, tag=f"delta{m}", name=f"delta{m}") for m in range(NB)]
    for m in range(NB):
        for tch in range(TCH):
            ps = p.ps_pool.tile([128, 512], F32, tag="mm", name="mm")
            nc.tensor.matmul(
                ps[:],
                w_dt[:, 128 * m:128 * (m + 1)],
                dbl[0:RNK, 512 * tch:512 * (tch + 1)],
                start=True, stop=True,
            )
            # softplus(s) = ln(1 + e^s) via the exp/ln table set
            spu = p.work_pool.tile([128, 512], F32, tag="spu", name="spu", bufs=1)
            _exp(p, nc.scalar.activation(
                spu[:], ps[:], AF.Exp, bias=con["dt_b"][m][:, 0:1]))
            _exp(p, nc.scalar.activation(
                delta[m][:, 512 * tch:512 * (tch + 1)], spu[:], AF.Ln, bias=1.0))
    return {"bc_dram": bc_dram, "delta": delta}


def _phase_cd(nc, p, cst, pre, rev, ten, phb, emit_out):
    xc, siluz = ten["xc"], ten["siluz"]
    bc_dram, delta = phb["bc_dram"], phb["delta"]
    con = _phase_a.consts[pre]

    w_out = [p.w_pool.tile([128, D], BF16, tag=f"w_out{k}", name=f"w_out{k}") for k in range(NB)]
    for k in range(NB):
        nc.sync.dma_start(w_out[k][:], cst[pre + "w_outT"][128 * k:128 * (k + 1), :])

    # --- phase C: selective scan + n-fold + gate ---
    yg = [p.big_pool.tile([128, L], BF16, tag=f"yg{m}", name=f"yg{m}") for m in range(NB)]
    for g in range(NB // 2):
        yp = [p.psy_pool.tile([128, L], F32, tag=f"yp{d2}", name=f"yp{d2}") for d2 in range(2)]
        dtx = [p.work_pool.tile([128, L], BF16, tag=f"dtx{d2}", name=f"dtx{d2}", bufs=1) for d2 in range(2)]
        xcdv = [p.work_pool.tile([128, L], BF16, tag=f"xcdv{d2}", name=f"xcdv{d2}", bufs=1) for d2 in range(2)]
        for d2 in range(2):
            m = 2 * g + d2
            nc.vector.tensor_tensor(dtx[d2][:], delta[m][:], xc[m][:], OP.mult)
            # Dv residual, accumulated into yp via an extra identity matmul
            nc.scalar.activation(xcdv[d2][:], xc[m][:], AF.Copy, scale=con["dv"][m][:, 0:1])
        for np2 in range(NST // 2):
            # two n-states packed per scan: segments [n0 | n0+1] along free,
            # with a zeroed da at each segment's first-processed element so
            # the recurrence resets (initial=0.0 covers the first segment)
            n0 = 2 * np2
            bpk = p.w_pool.tile([128, 2, L], BF16, tag=f"w_in{np2 % 2}", name="bpk", bufs=1)
            cpk = p.w_pool.tile([128, 2, L], BF16, tag=f"w_in{2 + np2 % 2}", name="cpk", bufs=1)
            nc.sync.dma_start(bpk[:], bc_dram[n0:n0 + 2, :].partition_broadcast(128))
            nc.sync.dma_start(
                cpk[:], bc_dram[NST + n0:NST + n0 + 2, :].partition_broadcast(128))
            for d2 in range(2):
                m = 2 * g + d2
                a_n0 = con["a_sb"][m][:, n0:n0 + 1]
                a_n1 = con["a_sb"][m][:, n0 + 1:n0 + 2]
                da = p.work_pool.tile([128, 2, L], BF16, tag="da", name="da")
                if rev:
                    nc.gpsimd.memset(da[:, 0, L - 1:L], 0.0)
                    _exp(p, nc.scalar.activation(
                        da[:, 0, 0:L - 1], delta[m][:, 0:L - 1], AF.Exp, scale=a_n0))
                    _exp(p, nc.scalar.activation(
                        da[:, 1, :], delta[m][:], AF.Exp, scale=a_n1))
                else:
                    nc.gpsimd.memset(da[:, 1, 0:1], 0.0)
                    _exp(p, nc.scalar.activation(
                        da[:, 0, :], delta[m][:], AF.Exp, scale=a_n0))
                    _exp(p, nc.scalar.activation(
                        da[:, 1, 1:L], delta[m][:, 1:L], AF.Exp, scale=a_n1))
                dbx = p.work_pool.tile([128, 2, L], BF16, tag="dbx", name="dbx")
                nc.vector.tensor_tensor(
                    dbx[:], dtx[d2][:].unsqueeze(1).broadcast_to([128, 2, L]),
                    bpk[:], OP.mult)
                h = p.work_pool.tile([128, 2, L], BF16, tag="h", name="h", bufs=1)
                h2 = h[:].rearrange("p a b -> p (a b)")
                da2 = da[:].rearrange("p a b -> p (a b)")
                dbx2 = dbx[:].rearrange("p a b -> p (a b)")
                if rev:
                    nc.vector.tensor_tensor_scan(
                        h2[:, ::-1], da2[:, ::-1], dbx2[:, ::-1], 0.0, OP.mult, OP.add)
                else:
                    nc.vector.tensor_tensor_scan(h2, da2, dbx2, 0.0, OP.mult, OP.add)
                hc = p.work_pool.tile([128, 2, L], BF16, tag="hc", name="hc")
                nc.vector.tensor_tensor(hc[:], h[:], cpk[:], OP.mult)
                for s in range(2):
                    for tch in range(TCH):
                        nc.tensor.matmul(
                            yp[d2][:, 512 * tch:512 * (tch + 1)],
                            p.ident[:],
                            hc[:, s, 512 * tch:512 * (tch + 1)],
                            start=(np2 == 0 and s == 0), stop=False,
                        )
        # gate: yg = (y + xc*Dv) * silu(z); the Dv term closes the PSUM group
        for d2 in range(2):
            m = 2 * g + d2
            for tch in range(TCH):
                nc.tensor.matmul(
                    yp[d2][:, 512 * tch:512 * (tch + 1)],
                    p.ident[:],
                    xcdv[d2][:, 512 * tch:512 * (tch + 1)],
                    start=False, stop=True,
                )
            # ACT copies PSUM->bf16 so the gate multiply runs in DVE 2x mode
            yv = p.work_pool.tile([128, L], BF16, tag=f"dtx{d2}", name="yv", bufs=1)
            nc.scalar.activation(yv[:], yp[d2][:], AF.Copy)
            nc.vector.tensor_tensor(yg[m][:], yv[:], siluz[m][:], OP.mult)

    # --- phase D: out_proj GEMM -> [t, D] PSUM tiles ---
    for m in range(TT):
        po = p.psd_pool.tile([128, D], F32, tag="po", name="po")
        for k in range(NB):
            nc.tensor.matmul(
                po[:], yg[k][:, 128 * m:128 * (m + 1)], w_out[k][:],
                start=(k == 0), stop=(k == NB - 1),
            )
        emit_out(m, po)


def build_program():
    nc = bacc.Bacc("TRN2", target_bir_lowering=False, debug=False)

    # Force exp/ln onto the one table set that has BOTH, so softplus
    # (exp then ln) doesn't ping-pong table loads. List order (= set ids)
    # is preserved; we only hide exp/ln from the other sets.
    import concourse.bacc as _bacc_mod
    from concourse.hw_specs import get_activation_tables as _gat

    def _patched_tables():
        tables = list(_gat(nc.m.arch).items())
        out = []
        for name, s in tables:
            if name != "natural_log_exp_and_others":
                s = s - {AF.Exp, AF.Ln}
            out.append((name, s))
        _bacc_mod._bass_rust.insert_act_table_loads(nc, out)

    nc.insert_act_table_loads = _patched_tables

    cst = {}
    cst["x_nat"] = nc.dram_tensor("x_nat", [L, D], F32, kind="ExternalInput")
    cst["xT"] = nc.dram_tensor("xT", [D, L], BF16, kind="ExternalInput")
    for pre in ("f_", "b_"):
        cst[pre + "w_inT"] = nc.dram_tensor(pre + "w_inT", [D, 2 * DI], BF16, kind="ExternalInput")
        cst[pre + "w_xT"] = nc.dram_tensor(pre + "w_xT", [DI, 64], BF16, kind="ExternalInput")
        cst[pre + "w_dtT"] = nc.dram_tensor(pre + "w_dtT", [RNK, DI], BF16, kind="ExternalInput")
        cst[pre + "w_outT"] = nc.dram_tensor(pre + "w_outT", [DI, D], BF16, kind="ExternalInput")
        cst[pre + "conv_w"] = nc.dram_tensor(pre + "conv_w", [DI, KCONV], F32, kind="ExternalInput")
        cst[pre + "conv_b"] = nc.dram_tensor(pre + "conv_b", [DI, 1], F32, kind="ExternalInput")
        cst[pre + "dt_b"] = nc.dram_tensor(pre + "dt_b", [DI, 1], F32, kind="ExternalInput")
        cst[pre + "A"] = nc.dram_tensor(pre + "A", [DI, NST], F32, kind="ExternalInput")
        cst[pre + "Dv"] = nc.dram_tensor(pre + "Dv", [DI, 1], F32, kind="ExternalInput")
    cst["ident"] = nc.dram_tensor("ident", [128, 128], BF16, kind="ExternalInput")
    cst["g_rep"] = nc.dram_tensor("g_rep", [128, D], F32, kind="ExternalInput")
    cst["bb_rep"] = nc.dram_tensor("bb_rep", [128, D], F32, kind="ExternalInput")
    out_d = nc.dram_tensor("out", [L, D], F32, kind="ExternalOutput")

    with tile.TileContext(nc) as tc:
        with (
            tc.tile_pool(name="io", bufs=1) as io_pool,
            tc.tile_pool(name="w", bufs=1) as w_pool,
            tc.tile_pool(name="big", bufs=1) as big_pool,
            tc.tile_pool(name="work", bufs=2) as work_pool,
            tc.tile_pool(name="s", bufs=1) as s_pool,
            tc.tile_pool(name="ps", bufs=2, space="PSUM") as ps_pool,
            tc.tile_pool(name="psy", bufs=1, space="PSUM") as psy_pool,
            tc.tile_pool(name="psd", bufs=2, space="PSUM") as psd_pool,
            tc.tile_pool(name="dram", bufs=1, space="DRAM") as dram_pool,
        ):
            p = P()
            p.io_pool, p.w_pool, p.big_pool, p.work_pool, p.s_pool = (
                io_pool, w_pool, big_pool, work_pool, s_pool)
            p.ps_pool, p.psy_pool, p.psd_pool, p.dram_pool = (
                ps_pool, psy_pool, psd_pool, dram_pool)

            p.sig_insts = []
            p.exp_insts = []
            # xT + f_ weights/consts first so phase A(f) starts ASAP; ident,
            # layernorm constants and b_ consts ride behind
            p.xT = [io_pool.tile([128, L], BF16, tag=f"xT{k}", name=f"xT{k}") for k in range(4)]
            for k in range(4):
                nc.sync.dma_start(p.xT[k][:], cst["xT"][128 * k:128 * (k + 1), :])
            _phase_a.consts = {"f_": _load_dir_consts(nc, p, cst, "f_")}
            ten_f = _phase_a(nc, p, cst, "f_", rev=False)
            phb_f = _phase_b(nc, p, cst, "f_", ten_f)
            _phase_a.consts["b_"] = _load_dir_consts(nc, p, cst, "b_")
            ten_b = _phase_a(nc, p, cst, "b_", rev=True)

            p.ident = io_pool.tile([128, 128], BF16, tag="ident", name="ident")
            nc.sync.dma_start(p.ident[:], cst["ident"][:])
            g_rep = io_pool.tile([128, D], F32, tag="g_rep", name="g_rep")
            bb_rep = io_pool.tile([128, D], F32, tag="bb_rep", name="bb_rep")
            nc.sync.dma_start(g_rep[:], cst["g_rep"][:])
            nc.sync.dma_start(bb_rep[:], cst["bb_rep"][:])
            eps_t = s_pool.tile([128, 1], F32, tag="eps_t", name="eps_t")
            nc.gpsimd.memset(eps_t[:], LN_EPS)

            outf = [io_pool.tile([128, D], BF16, tag=f"outf{m}", name=f"outf{m}") for m in range(TT)]

            def emit_f(m, po):
                nc.scalar.activation(outf[m][:], po[:], AF.Copy)

            def emit_b(m, po):
                # combine (f + b)/2 + x, then layernorm over D, then store;
                # bufs=2 so consecutive time tiles pipeline instead of
                # serializing on buffer reuse
                xnat = io_pool.tile([128, D], F32, tag="xnat", name="xnat", bufs=2)
                nc.sync.dma_start(xnat[:], cst["x_nat"][128 * m:128 * (m + 1), :])
                pre_f = io_pool.tile([128, D], F32, tag="pre_f", name="pre_f", bufs=2)
                nc.vector.tensor_tensor(pre_f[:], outf[m][:], xnat[:], OP.add)
                o = io_pool.tile([128, D], F32, tag="o_comb", name="o_comb", bufs=2)
                mu_raw = s_pool.tile([128, 1], F32, tag="mu_raw", name="mu_raw", bufs=2)
                nc.vector.scalar_tensor_tensor(
                    o[:], po[:], 1.0, pre_f[:], OP.mult, OP.add, accum_out=mu_raw[:]
                )
                # mean subtract + variance + rstd all on ACT: xm = o - mu_raw/D,
                # lv = ln(var_raw/D + eps), rstd = exp(-lv/2)
                mu_neg = s_pool.tile([128, 1], F32, tag="mu", name="mu_neg", bufs=2)
                nc.scalar.mul(mu_neg[:], mu_raw[:], -1.0 / D)
                xm = io_pool.tile([128, D], F32, tag="xm", name="xm", bufs=2)
                nc.scalar.activation(xm[:], o[:], AF.Identity, bias=mu_neg[:, 0:1])
                sqd = io_pool.tile([128, D], F32, tag="pre_f", name="sqd", bufs=2)
                var_raw = s_pool.tile([128, 1], F32, tag="var_raw", name="var_raw", bufs=2)
                nc.scalar.activation(sqd[:], xm[:], AF.Square, accum_out=var_raw[:])
                lv = s_pool.tile([128, 1], F32, tag="lv", name="lv", bufs=2)
                _exp(p, nc.scalar.activation(
                    lv[:], var_raw[:], AF.Ln, scale=1.0 / D, bias=eps_t[:, 0:1]))
                rstd = s_pool.tile([128, 1], F32, tag="rstd", name="rstd", bufs=2)
                _exp(p, nc.scalar.activation(rstd[:], lv[:], AF.Exp, scale=-0.5))
                o1 = io_pool.tile([128, D], F32, tag="o_comb", name="o1", bufs=2)
                nc.vector.scalar_tensor_tensor(
                    o1[:], xm[:], rstd[:, 0:1], g_rep[:], OP.mult, OP.mult
                )
                o2 = io_pool.tile([128, D], F32, tag="xnat", name="o2", bufs=2)
                nc.vector.tensor_tensor(o2[:], o1[:], bb_rep[:], OP.add)
                nc.sync.dma_start(out_d[128 * m:128 * (m + 1), :], o2[:])

            _phase_cd(nc, p, cst, "f_", rev=False, ten=ten_f, phb=phb_f, emit_out=emit_f)
            phb_b = _phase_b(nc, p, cst, "b_", ten_b)
            _phase_cd(nc, p, cst, "b_", rev=True, ten=ten_b, phb=phb_b, emit_out=emit_b)

    nc.compile()
    return nc


_CACHE = {}


def _host_inputs(inputs):
    """Per-core input maps from the full problem inputs."""
    x = np.asarray(inputs["x"], np.float32)
    base = {}
    for pre in ("f_", "b_"):
        base[pre + "w_inT"] = np.ascontiguousarray(
            np.asarray(inputs[pre + "in_proj"], np.float32).T
        ).astype(nbf)
        base[pre + "w_xT"] = np.ascontiguousarray(
            np.asarray(inputs[pre + "x_proj"], np.float32).T
        ).astype(nbf)
        base[pre + "w_dtT"] = np.ascontiguousarray(
            np.asarray(inputs[pre + "dt_w"], np.float32).T
        ).astype(nbf)
        base[pre + "w_outT"] = np.ascontiguousarray(
            0.5 * np.asarray(inputs[pre + "out_proj"], np.float32).T
        ).astype(nbf)
        base[pre + "conv_w"] = np.asarray(inputs[pre + "conv_w"], np.float32)
        base[pre + "conv_b"] = np.asarray(inputs[pre + "conv_b"], np.float32).reshape(DI, 1)
        base[pre + "dt_b"] = np.asarray(inputs[pre + "dt_b"], np.float32).reshape(DI, 1)
        base[pre + "A"] = -np.exp(np.asarray(inputs[pre + "A_log"], np.float32))
        base[pre + "Dv"] = np.asarray(inputs[pre + "Dv"], np.float32).reshape(DI, 1)
    base["ident"] = np.eye(128, dtype=nbf)
    base["g_rep"] = np.broadcast_to(np.asarray(inputs["ln_g"], np.float32), (128, D)).copy()
    base["bb_rep"] = np.broadcast_to(np.asarray(inputs["ln_b"], np.float32), (128, D)).copy()

    in_maps = []
    for i in range(B):
        m = dict(base)
        m["x_nat"] = np.ascontiguousarray(x[i])
        m["xT"] = np.ascontiguousarray(x[i].T).astype(nbf)
        in_maps.append(m)
    return in_maps


def kernel(**inputs):
    if "nc" not in _CACHE:
        _CACHE["nc"] = build_program()
    nc = _CACHE["nc"]
    in_maps = _host_inputs(inputs)
    res = run_bass_kernel_spmd(nc, in_maps, core_ids=list(range(B)))
    out = np.stack([res.results[i]["out"] for i in range(B)], axis=0)
    return out.astype(np.float32)

